# revision 8
# baseline (speedup 1.0000x reference)
"""Trainium2 Bass kernel for nn_CrossAttention_31078383354530.

Reference computation (b=2, n=m=2048, qd=1024, cd=768, heads=8, dh=128):
    q = x @ Wq; k = ctx @ Wk; v = ctx @ Wv  (split into 8 heads of 128)
    sim = (q @ k^T) * dh**-0.5 over the FLATTENED (b*n)=4096 token axis
    attn = softmax((sim - mean)*1.5 + mean) == softmax(1.5*scale*(q@k^T))
        exactly (the mean-centering is a per-row constant shift)
    out = attn @ v -> merge heads -> y = out @ Wout + bout

Sharding (8 cores): context-token-sharded K/V projection + pipelined
AllGathers of bf16 K/V (2 heads per collective), then each core runs all 8
heads' attention for its own 512-query-token slice and its own final
projection -> the output is a disjoint row-slice per core.

Performance structure:
  * A tiny warm-up AllGather is triggered first thing: the CC cores have a
    large fixed startup latency (~70us to first mesh in cold traces), and
    the dummy collective absorbs it so AG0's mesh can begin as soon as its
    data is ready.
  * Wk/Wv are DMA'd as per-pair column slices, ordered pair-0-first, so
    pair 0's K/V projections (and therefore the AG0 trigger) run ~12us in.
  * sim runs 2 groups ahead of PV/rowsum in issue order so the PE never
    head-of-line blocks on the Exp activation. A continuously-busy PE holds
    the 2.4 GHz DVFS state (idle gaps demote the clock).
  * PV uses full 512-column moving streams (lhsT = V_j): short-stream
    variants are LD_WEIGHTS-bound on real HW. The softmax rowsum is a
    ones-stationary matmul on the same at stream; normalization runs on
    DVE/GpSimd off the PE critical path.
  * All weights (Wq/x/Wout) prefetched up front; Q projections for heads
    2..7 are interleaved into the attention stream to fill PE slack.
"""

import sys

if "/opt/trn_rl_repo" not in sys.path:
    sys.path.insert(0, "/opt/trn_rl_repo")

import ml_dtypes
import numpy as np

import concourse.bass as bass  # noqa: F401
import concourse.mybir as mybir
import concourse.tile as tile
from concourse import bacc, bass_utils

F32 = mybir.dt.float32
BF16 = mybir.dt.bfloat16
AF = mybir.ActivationFunctionType

P = 128
N_CORES = 8
HEADS = 8
DH = 128
TOK = 4096             # b*n flattened token axis (attention mixes batches!)
SLICE = TOK // N_CORES  # 512 tokens per core
QD = 1024
CD = 768
INNER = 1024
KC = QD // P           # 8 qd chunks
CC = CD // P           # 6 cd chunks
JT = TOK // P          # 32 j-tiles per head
NG = JT // 2           # 16 sim/exp groups per head (2 j-tiles per group)
TAU_SCALE = 1.5 * (DH ** -0.5)

_CACHE = {}


def _build():
    nc = bacc.Bacc(num_devices=N_CORES)

    xTs = nc.declare_dram_parameter("xTs", [QD, SLICE], BF16, isOutput=False)
    cTs = nc.declare_dram_parameter("cTs", [CD, SLICE], BF16, isOutput=False)
    Wq = nc.declare_dram_parameter("Wq", [QD, INNER], BF16, isOutput=False)
    Wk = nc.declare_dram_parameter("Wk", [CD, INNER], BF16, isOutput=False)
    Wv = nc.declare_dram_parameter("Wv", [CD, INNER], BF16, isOutput=False)
    Wout = nc.declare_dram_parameter("Wout", [INNER, QD], BF16, isOutput=False)
    boutT = nc.declare_dram_parameter("boutT", [P, KC], F32, isOutput=False)
    yT = nc.declare_dram_parameter("yT", [KC, P, SLICE], F32, isOutput=True)

    with tile.TileContext(nc) as tc:
        with (
            tc.tile_pool(name="const", bufs=1) as const,
            tc.tile_pool(name="sb", bufs=1) as sb,
            tc.tile_pool(name="ps", bufs=1, space="PSUM") as ps,
            tc.tile_pool(name="dram", bufs=1, space="DRAM") as dram,
        ):
            NP = HEADS // 2  # head pairs; one AllGather per pair
            kv_in = [dram.tile([2, 2, P, SLICE], BF16, name=f"kv_in{p}")
                     for p in range(NP)]
            kv_g = [dram.tile([N_CORES, 2, 2, P, SLICE], BF16,
                              addr_space="Shared", name=f"kv_g{p}")
                    for p in range(NP)]

            # ---- warm-up collective: absorb the CC cores' fixed startup
            # latency so AG0's mesh can begin as soon as its data is ready
            warm_in = dram.tile([1, P], BF16, name="warm_in")
            warm_g = dram.tile([N_CORES, 1, P], BF16, addr_space="Shared",
                               name="warm_g")
            warm_sb = const.tile([1, P], BF16, name="warm_sb")
            nc.vector.memset(warm_sb[:], 0.0)
            nc.sync.dma_start(warm_in[:], warm_sb[:])
            nc.gpsimd.collective_compute(
                "AllGather", mybir.AluOpType.bypass,
                replica_groups=[list(range(N_CORES))],
                ins=[warm_in.opt()], outs=[warm_g.opt()],
            )

            ones_b = const.tile([P, 1], BF16, name="ones_b")
            nc.vector.memset(ones_b[:], 1.0)
            bout_sb = const.tile([P, KC], F32, name="bout_sb")
            nc.sync.dma_start(bout_sb[:], boutT[:, :])

            # ---- input DMAs: pair-0 K/V weight slices first so pair 0's
            # projections (and AG0) launch early; everything else behind
            cts = []
            for k in range(CC):
                t = sb.tile([P, SLICE], BF16, name=f"cts{k}", tag="cts", bufs=CC)
                nc.sync.dma_start(t[:], cTs[k * P:(k + 1) * P, :])
                cts.append(t)
            # per-pair column slices of Wk / Wv: wk_p[p][k], wv_p[p][k]
            wk_p = [[None] * CC for _ in range(NP)]
            wv_p = [[None] * CC for _ in range(NP)]

            def load_pair_weights(p):
                for k in range(CC):
                    t = sb.tile([P, 2 * DH], BF16, name=f"wk{p}_{k}",
                                tag="wkp", bufs=NP * CC)
                    nc.sync.dma_start(
                        t[:], Wk[k * P:(k + 1) * P,
                                 p * 2 * DH:(p + 1) * 2 * DH])
                    wk_p[p][k] = t
                for k in range(CC):
                    t = sb.tile([P, 2 * DH], BF16, name=f"wv{p}_{k}",
                                tag="wvp", bufs=NP * CC)
                    nc.sync.dma_start(
                        t[:], Wv[k * P:(k + 1) * P,
                                 p * 2 * DH:(p + 1) * 2 * DH])
                    wv_p[p][k] = t

            load_pair_weights(0)
            load_pair_weights(1)
            xts = []
            for k in range(KC):
                t = sb.tile([P, SLICE], BF16, name=f"xts{k}", tag="xts", bufs=KC)
                nc.sync.dma_start(t[:], xTs[k * P:(k + 1) * P, :])
                xts.append(t)
            wqt = []
            for k in range(KC):
                t = sb.tile([P, INNER], BF16, name=f"wqt{k}", tag="wqt", bufs=KC)
                nc.sync.dma_start(t[:], Wq[k * P:(k + 1) * P, :])
                wqt.append(t)
            load_pair_weights(2)
            load_pair_weights(3)
            wo = []
            for c in range(KC):
                t = sb.tile([P, KC, DH], BF16, name=f"wo{c}", tag="wo", bufs=KC)
                nc.sync.dma_start(
                    t[:],
                    Wout.ap()[:, c * DH:(c + 1) * DH].rearrange(
                        "(k p) c -> p k c", p=P),
                )
                wo.append(t)

            # ---- per-pair K-proj, V-proj, AllGather ----
            for p in range(NP):
                kps = ps.tile([P, 2, SLICE], F32, name=f"kps{p}", tag="sim",
                              bufs=3)
                for hl in range(2):
                    for k in range(CC):
                        nc.tensor.matmul(kps[:, hl, :],
                                         wk_p[p][k][:, hl * DH:(hl + 1) * DH],
                                         cts[k][:],
                                         start=(k == 0), stop=(k == CC - 1))
                ksb = sb.tile([P, 2, SLICE], BF16, name=f"ksb{p}", tag="ksb",
                              bufs=2)
                nc.vector.tensor_copy(ksb[:], kps[:])
                for hl in range(2):
                    nc.sync.dma_start(kv_in[p][0, hl], ksb[:, hl, :])

                vps = ps.tile([P, 4, 2 * DH], F32, name=f"vps{p}", tag="sim",
                              bufs=3)
                for tt in range(4):
                    for k in range(CC):
                        nc.tensor.matmul(
                            vps[:, tt, :],
                            cts[k][:, tt * P:(tt + 1) * P],
                            wv_p[p][k][:],
                            start=(k == 0), stop=(k == CC - 1))
                vsbq = sb.tile([P, 4, 2 * DH], BF16, name=f"vsbq{p}",
                               tag="vsbq", bufs=2)
                nc.vector.tensor_copy(vsbq[:], vps[:])
                for hl in range(2):
                    # kv_in V layout: [128 tok%128, (tt, dh)]
                    nc.sync.dma_start(kv_in[p][1, hl],
                                      vsbq[:, :, hl * DH:(hl + 1) * DH])
                nc.gpsimd.collective_compute(
                    "AllGather", mybir.AluOpType.bypass,
                    replica_groups=[list(range(N_CORES))],
                    ins=[kv_in[p].opt()], outs=[kv_g[p].opt()],
                )

            # ---- Q projection (heads 0,1 up front; h+2 interleaved) ----
            qsb = [None] * HEADS

            def qproj(h):
                qps = ps.tile([P, 2, SLICE], F32, name=f"qps{h}", tag="sim",
                              bufs=3)
                for k in range(KC):
                    nc.tensor.matmul(qps[:, 0, :],
                                     wqt[k][:, h * DH:(h + 1) * DH],
                                     xts[k][:],
                                     start=(k == 0), stop=(k == KC - 1))
                t = sb.tile([P, SLICE], BF16, name=f"qsb{h}", tag="qsb",
                            bufs=HEADS)
                nc.vector.tensor_copy(t[:], qps[:, 0, :])
                qsb[h] = t

            qproj(0)
            qproj(1)

            # ---- attention: sim issued 2 groups ahead of PV/rowsum ----
            osb = [None] * HEADS
            for h in range(HEADS):
                p, hl = h // 2, h % 2
                kh = sb.tile([P, TOK], BF16, name=f"kh{h}", tag="kh", bufs=2)
                vh = sb.tile([P, TOK], BF16, name=f"vh{h}", tag="vh", bufs=2)
                for r in range(N_CORES):
                    nc.sync.dma_start(kh[:, r * SLICE:(r + 1) * SLICE],
                                      kv_g[p][r, 0, hl])
                    nc.sync.dma_start(vh[:, r * SLICE:(r + 1) * SLICE],
                                      kv_g[p][r, 1, hl])

                pv_ps = ps.tile([P, SLICE], F32, name=f"pv{h}", tag="pv",
                                bufs=1)
                rs_ps = ps.tile([1, SLICE], F32, name=f"rs{h}", tag="rs",
                                bufs=1)
                ats = [None] * NG
                for g in range(NG + 2):
                    if g < NG:
                        sim_ps = ps.tile([P, 2, SLICE], F32,
                                         name=f"sim{h}_{g}", tag="sim", bufs=3)
                        for jj in range(2):
                            j = 2 * g + jj
                            nc.tensor.matmul(sim_ps[:, jj, :],
                                             kh[:, j * P:(j + 1) * P],
                                             qsb[h][:],
                                             start=True, stop=True)
                        at = sb.tile([P, 2, SLICE], BF16, name=f"at{h}_{g}",
                                     tag="at", bufs=4)
                        nc.scalar.activation(at[:], sim_ps[:], AF.Exp,
                                             scale=TAU_SCALE)
                        ats[g] = at
                    if g == 10 and h + 2 < HEADS:
                        qproj(h + 2)
                    if g >= 2:
                        gg = g - 2
                        at = ats[gg]
                        for jj in range(2):
                            j = 2 * gg + jj
                            nc.tensor.matmul(pv_ps[:],
                                             vh[:, j * P:(j + 1) * P],
                                             at[:, jj, :],
                                             start=(j == 0), stop=(j == JT - 1))
                            nc.tensor.matmul(rs_ps[:], ones_b[:],
                                             at[:, jj, :],
                                             start=(j == 0), stop=(j == JT - 1))

                # normalization entirely off the PE critical path
                pvc = sb.tile([P, SLICE], F32, name=f"pvsb{h}", tag="pvsb",
                              bufs=2)
                nc.vector.tensor_copy(pvc[:], pv_ps[:])
                rsc = sb.tile([1, SLICE], F32, name=f"rssb{h}", tag="rssb",
                              bufs=2)
                nc.vector.tensor_copy(rsc[:], rs_ps[:])
                recip = sb.tile([1, SLICE], F32, name=f"recip{h}", tag="recip",
                                bufs=2)
                nc.vector.reciprocal(recip[:], rsc[:])
                bc = sb.tile([P, SLICE], F32, name=f"bc{h}", tag="bc", bufs=2)
                nc.gpsimd.partition_broadcast(bc[:], recip[:])
                ot = sb.tile([P, SLICE], BF16, name=f"osb{h}", tag="osb",
                             bufs=HEADS)
                nc.vector.tensor_tensor(ot[:], pvc[:], bc[:],
                                        mybir.AluOpType.mult)
                osb[h] = ot

            # ---- final projection: yT[cc] = Wout[:, cc]^T @ out^T + bout --
            for c in range(KC):
                yps = ps.tile([P, SLICE], F32, name=f"yps{c}",
                              tag=("pv" if c % 2 == 0 else "rs"), bufs=1)
                for ic in range(HEADS):
                    nc.tensor.matmul(yps[:], wo[c][:, ic, :], osb[ic][:],
                                     start=(ic == 0), stop=(ic == HEADS - 1))
                yt = sb.tile([P, SLICE], F32, name=f"yt{c}", tag="yt", bufs=2)
                nc.scalar.activation(yt[:], yps[:], AF.Identity,
                                     bias=bout_sb[:, c:c + 1], scale=1.0)
                nc.sync.dma_start(yT.ap()[c], yt[:])

    nc.compile()
    return nc


def _get_nc():
    if "nc" not in _CACHE:
        _CACHE["nc"] = _build()
    return _CACHE["nc"]


def _bf16(a):
    return np.ascontiguousarray(np.asarray(a, np.float32).astype(ml_dtypes.bfloat16))


def _prep_in_maps(x, context, Wq, Wk, Wv, Wout, bout):
    x_f = np.asarray(x, dtype=np.float32).reshape(TOK, QD)
    c_f = np.asarray(context, dtype=np.float32).reshape(TOK, CD)
    Wq = _bf16(Wq)
    Wk = _bf16(Wk)
    Wv = _bf16(Wv)
    Wout = _bf16(Wout)
    boutT = np.ascontiguousarray(
        np.asarray(bout, dtype=np.float32).reshape(KC, P).T)
    in_maps = []
    for c in range(N_CORES):
        sl = slice(c * SLICE, (c + 1) * SLICE)
        in_maps.append({
            "xTs": _bf16(x_f[sl].T),
            "cTs": _bf16(c_f[sl].T),
            "Wq": Wq, "Wk": Wk, "Wv": Wv, "Wout": Wout, "boutT": boutT,
        })
    return in_maps


def _assemble(results):
    y = np.empty((TOK, QD), dtype=np.float32)
    for c in range(N_CORES):
        yt = results[c]["yT"]   # [KC, P, SLICE]
        y[c * SLICE:(c + 1) * SLICE] = (
            yt.transpose(2, 0, 1).reshape(SLICE, QD))
    return y.reshape(2, TOK // 2, QD)


def run(inputs, trace=False, **kw):
    nc = _get_nc()
    in_maps = _prep_in_maps(**inputs)
    res = bass_utils.run_bass_kernel_spmd(
        nc, in_maps, core_ids=list(range(N_CORES)), trace=trace, **kw)
    return _assemble(res.results), res


def kernel(**inputs):
    out, _ = run(inputs, trace=False)
    return out


# revision 9
# speedup vs baseline: 1.5106x; 1.5106x over previous
"""Trainium2 Bass kernel for nn_CrossAttention_31078383354530.

Reference computation (b=2, n=m=2048, qd=1024, cd=768, heads=8, dh=128):
    q = x @ Wq; k = ctx @ Wk; v = ctx @ Wv  (split into 8 heads of 128)
    sim = (q @ k^T) * dh**-0.5 over the FLATTENED (b*n)=4096 token axis
    attn = softmax((sim - mean)*1.5 + mean) == softmax(1.5*scale*(q@k^T))
        exactly (the mean-centering is a per-row constant shift)
    out = attn @ v -> merge heads -> y = out @ Wout + bout

Sharding (8 cores): context-token-sharded K/V projection + pipelined
AllGathers of bf16 K/V (2 heads per collective), then each core runs all 8
heads' attention for its own 512-query-token slice and its own final
projection -> the output is a disjoint row-slice per core.

Key performance structure (vs the v0 kernel):
  * PV is computed TRANSPOSED: out[q, dh+1] = at^T @ [V | ones], so the
    softmax denominator is column 128 of the same 129-column matmul stream
    instead of a second full-width ones-matmul pass over `at` (which cost
    an extra 512 cols/j-tile of PE streaming).
  * The softmax normalize becomes a per-partition tensor_scalar on DVE
    (denominator lives on the partition axis), then 4 cheap PE transposes
    per head restore [dh, q] for the output projection.
  * sim runs 2 groups ahead of PV in issue order so the PE never head-of-
    line blocks on the Exp activation; transposes for head h are emitted
    under head h+1's sim work. A continuously-busy PE holds the 2.4 GHz
    DVFS state (idle gaps demote the clock to 1.2-2.0 GHz).
  * Per-pair K-proj -> V-proj -> AllGather so AG0 launches ~25us in; Wq/x/
    Wout tiles are all prefetched at kernel start.
"""

import sys

if "/opt/trn_rl_repo" not in sys.path:
    sys.path.insert(0, "/opt/trn_rl_repo")

import ml_dtypes
import numpy as np

import concourse.bass as bass  # noqa: F401
import concourse.mybir as mybir
import concourse.tile as tile
from concourse import bacc, bass_utils, masks

F32 = mybir.dt.float32
BF16 = mybir.dt.bfloat16
AF = mybir.ActivationFunctionType

P = 128
N_CORES = 8
HEADS = 8
DH = 128
TOK = 4096             # b*n flattened token axis (attention mixes batches!)
SLICE = TOK // N_CORES  # 512 tokens per core
QD = 1024
CD = 768
INNER = 1024
KC = QD // P           # 8 qd chunks
CC = CD // P           # 6 cd chunks
JT = TOK // P          # 32 j-tiles per head
NG = JT // 2           # 16 sim/exp groups per head (2 j-tiles per group)
TAU_SCALE = 1.5 * (DH ** -0.5)

_CACHE = {}


def _build():
    nc = bacc.Bacc(num_devices=N_CORES)

    xTs = nc.declare_dram_parameter("xTs", [QD, SLICE], BF16, isOutput=False)
    cTs = nc.declare_dram_parameter("cTs", [CD, SLICE], BF16, isOutput=False)
    Wq = nc.declare_dram_parameter("Wq", [QD, INNER], BF16, isOutput=False)
    Wk = nc.declare_dram_parameter("Wk", [CD, INNER], BF16, isOutput=False)
    Wv = nc.declare_dram_parameter("Wv", [CD, INNER], BF16, isOutput=False)
    Wout = nc.declare_dram_parameter("Wout", [INNER, QD], BF16, isOutput=False)
    boutT = nc.declare_dram_parameter("boutT", [P, KC], F32, isOutput=False)
    yT = nc.declare_dram_parameter("yT", [KC, P, SLICE], F32, isOutput=True)

    with tile.TileContext(nc) as tc:
        with (
            tc.tile_pool(name="const", bufs=1) as const,
            tc.tile_pool(name="sb", bufs=1) as sb,
            tc.tile_pool(name="ps", bufs=1, space="PSUM") as ps,
            tc.tile_pool(name="dram", bufs=1, space="DRAM") as dram,
        ):
            NP = HEADS // 2  # head pairs; one AllGather per pair
            kv_in = [dram.tile([2, 2, P, SLICE], BF16, name=f"kv_in{p}")
                     for p in range(NP)]
            kv_g = [dram.tile([N_CORES, 2, 2, P, SLICE], BF16,
                              addr_space="Shared", name=f"kv_g{p}")
                    for p in range(NP)]

            ident = const.tile([P, P], BF16, name="ident")
            masks.make_identity(nc, ident[:])
            bout_sb = const.tile([P, KC], F32, name="bout_sb")
            nc.sync.dma_start(bout_sb[:], boutT[:, :])

            # ---- prefetch ALL inputs up front (K/V proj first, Q + out
            # projection weights behind them so the PE never waits on DMA)
            cts = []
            for k in range(CC):
                t = sb.tile([P, SLICE], BF16, name=f"cts{k}", tag="cts", bufs=CC)
                nc.sync.dma_start(t[:], cTs[k * P:(k + 1) * P, :])
                cts.append(t)
            wkt = []
            for k in range(CC):
                t = sb.tile([P, INNER], BF16, name=f"wkt{k}", tag="wkt", bufs=CC)
                nc.sync.dma_start(t[:], Wk[k * P:(k + 1) * P, :])
                wkt.append(t)
            wvt = []
            for k in range(CC):
                t = sb.tile([P, INNER], BF16, name=f"wvt{k}", tag="wvt", bufs=CC)
                nc.sync.dma_start(t[:], Wv[k * P:(k + 1) * P, :])
                wvt.append(t)
            xts = []
            for k in range(KC):
                t = sb.tile([P, SLICE], BF16, name=f"xts{k}", tag="xts", bufs=KC)
                nc.sync.dma_start(t[:], xTs[k * P:(k + 1) * P, :])
                xts.append(t)
            wqt = []
            for k in range(KC):
                t = sb.tile([P, INNER], BF16, name=f"wqt{k}", tag="wqt", bufs=KC)
                nc.sync.dma_start(t[:], Wq[k * P:(k + 1) * P, :])
                wqt.append(t)
            wo = []
            for c in range(KC):
                t = sb.tile([P, KC, DH], BF16, name=f"wo{c}", tag="wo", bufs=KC)
                nc.sync.dma_start(
                    t[:],
                    Wout.ap()[:, c * DH:(c + 1) * DH].rearrange(
                        "(k p) c -> p k c", p=P),
                )
                wo.append(t)

            # ---- per-pair K-proj, V-proj, AllGather (AG0 launches early) --
            for p in range(NP):
                kps = ps.tile([P, 2, SLICE], F32, name=f"kps{p}", tag="sim",
                              bufs=2)
                for hl in range(2):
                    h = 2 * p + hl
                    for k in range(CC):
                        nc.tensor.matmul(kps[:, hl, :],
                                         wkt[k][:, h * DH:(h + 1) * DH],
                                         cts[k][:],
                                         start=(k == 0), stop=(k == CC - 1))
                ksb = sb.tile([P, 2, SLICE], BF16, name=f"ksb{p}", tag="ksb",
                              bufs=2)
                nc.vector.tensor_copy(ksb[:], kps[:])
                for hl in range(2):
                    nc.sync.dma_start(kv_in[p][0, hl], ksb[:, hl, :])

                vps = ps.tile([P, 4, 2 * DH], F32, name=f"vps{p}", tag="sim",
                              bufs=2)
                for tt in range(4):
                    for k in range(CC):
                        nc.tensor.matmul(
                            vps[:, tt, :],
                            cts[k][:, tt * P:(tt + 1) * P],
                            wvt[k][:, p * 2 * DH:(p + 1) * 2 * DH],
                            start=(k == 0), stop=(k == CC - 1))
                vsbq = sb.tile([P, 4, 2 * DH], BF16, name=f"vsbq{p}",
                               tag="vsbq", bufs=2)
                nc.vector.tensor_copy(vsbq[:], vps[:])
                for hl in range(2):
                    # kv_in V layout: [128 tok%128, (tt, dh)]
                    nc.sync.dma_start(kv_in[p][1, hl],
                                      vsbq[:, :, hl * DH:(hl + 1) * DH])
                nc.gpsimd.collective_compute(
                    "AllGather", mybir.AluOpType.bypass,
                    replica_groups=[list(range(N_CORES))],
                    ins=[kv_in[p].opt()], outs=[kv_g[p].opt()],
                )

            # ---- Q projection helper (heads 0,1 up front; h+2 interleaved
            # into head h's attention stream) ----
            qsb = [None] * HEADS

            def qproj(h):
                qps = ps.tile([P, 2, SLICE], F32, name=f"qps{h}", tag="sim",
                              bufs=2)
                for k in range(KC):
                    nc.tensor.matmul(qps[:, 0, :],
                                     wqt[k][:, h * DH:(h + 1) * DH],
                                     xts[k][:],
                                     start=(k == 0), stop=(k == KC - 1))
                t = sb.tile([P, SLICE], BF16, name=f"qsb{h}", tag="qsb",
                            bufs=HEADS)
                nc.vector.tensor_copy(t[:], qps[:, 0, :])
                qsb[h] = t

            for h in range(HEADS):
                qproj(h)

            # ---- attention: per head, sim runs 2 groups ahead of PV^T ----
            osb = [None] * HEADS
            pending = []  # deferred transpose work from the previous head

            def flush_pending():
                while pending:
                    fn = pending.pop(0)
                    fn()

            for h in range(HEADS):
                p, hl = h // 2, h % 2
                kh = sb.tile([P, TOK], BF16, name=f"kh{h}", tag="kh", bufs=2)
                vext = sb.tile([P, JT, DH + 1], BF16, name=f"vext{h}",
                               tag="vext", bufs=2)
                nc.vector.memset(vext[:, :, DH:DH + 1], 1.0)
                for r in range(N_CORES):
                    nc.sync.dma_start(kh[:, r * SLICE:(r + 1) * SLICE],
                                      kv_g[p][r, 0, hl])
                    nc.sync.dma_start(vext[:, r * 4:(r + 1) * 4, 0:DH],
                                      kv_g[p][r, 1, hl])

                pv = [None] * 4  # one psum bank per qt: a bank supports only
                # ONE open accumulation group at a time (two concurrently
                # open groups in a bank corrupt the first-opened one)
                ats = [None] * NG
                for g in range(NG + 2):
                    if g < NG:
                        sim_ps = ps.tile([P, 2, SLICE], F32,
                                         name=f"sim{h}_{g}", tag="sim", bufs=2)
                        for jj in range(2):
                            j = 2 * g + jj
                            nc.tensor.matmul(sim_ps[:, jj, :],
                                             kh[:, j * P:(j + 1) * P],
                                             qsb[h][:],
                                             start=True, stop=True)
                        at = sb.tile([P, 2, SLICE], BF16, name=f"at{h}_{g}",
                                     tag="at", bufs=4)
                        nc.scalar.activation(at[:], sim_ps[:], AF.Exp,
                                             scale=TAU_SCALE)
                        ats[g] = at
                    if g == 1:
                        flush_pending()
                    if g >= 2:
                        gg = g - 2
                        if gg == 0:
                            for qt in range(4):
                                pv[qt] = ps.tile([P, SLICE], F32,
                                                 name=f"pv{h}_{qt}", tag="pv",
                                                 bufs=4)
                        at = ats[gg]
                        for jj in range(2):
                            j = 2 * gg + jj
                            for qt in range(4):
                                nc.tensor.matmul(
                                    pv[qt][:, 0:DH + 1],
                                    at[:, jj, qt * P:(qt + 1) * P],
                                    vext[:, j, :],
                                    start=(j == 0), stop=(j == JT - 1))

                # head end: reciprocal + per-partition normalize (DVE), then
                # PE transposes deferred under head h+1's first sim groups
                rcp = sb.tile([P, 4], F32, name=f"rcp{h}", tag="rcp",
                              bufs=2)
                osbT = sb.tile([P, 4, DH], BF16, name=f"osbT{h}", tag="osbT",
                               bufs=2)
                for qt in range(4):
                    nc.vector.reciprocal(rcp[:, qt:qt + 1],
                                         pv[qt][:, DH:DH + 1])
                for qt in range(4):
                    nc.vector.tensor_scalar(
                        osbT[:, qt, :], pv[qt][:, 0:DH],
                        rcp[:, qt:qt + 1], None,
                        mybir.AluOpType.mult)

                def make_xpose(h=h, osbT=osbT):
                    def xpose():
                        xp = ps.tile([P, 4, 2 * DH], BF16, name=f"xp{h}",
                                     tag="pv", bufs=4)
                        for qt in range(4):
                            nc.tensor.transpose(xp[:, qt, 0:DH],
                                                osbT[:, qt, :], ident[:])
                        ot = sb.tile([P, 4, DH], BF16, name=f"osb{h}",
                                     tag="osb", bufs=HEADS)
                        nc.vector.tensor_copy(ot[:], xp[:, :, 0:DH])
                        osb[h] = ot
                    return xpose

                pending.append(make_xpose())
                if h == HEADS - 1:
                    flush_pending()

            # ---- final projection: yT[cc] = Wout[:, cc]^T @ out^T + bout --
            for c in range(KC):
                yps = ps.tile([P, SLICE], F32, name=f"yps{c}", tag="pv",
                              bufs=4)
                for ic in range(HEADS):
                    nc.tensor.matmul(yps[:], wo[c][:, ic, :], osb[ic][:],
                                     start=(ic == 0), stop=(ic == HEADS - 1))
                yt = sb.tile([P, SLICE], F32, name=f"yt{c}", tag="yt", bufs=2)
                nc.scalar.activation(yt[:], yps[:], AF.Identity,
                                     bias=bout_sb[:, c:c + 1], scale=1.0)
                nc.sync.dma_start(yT.ap()[c], yt[:])

    nc.compile()
    return nc


def _get_nc():
    if "nc" not in _CACHE:
        _CACHE["nc"] = _build()
    return _CACHE["nc"]


def _bf16(a):
    return np.ascontiguousarray(np.asarray(a, np.float32).astype(ml_dtypes.bfloat16))


def _prep_in_maps(x, context, Wq, Wk, Wv, Wout, bout):
    x_f = np.asarray(x, dtype=np.float32).reshape(TOK, QD)
    c_f = np.asarray(context, dtype=np.float32).reshape(TOK, CD)
    Wq = _bf16(Wq)
    Wk = _bf16(Wk)
    Wv = _bf16(Wv)
    Wout = _bf16(Wout)
    boutT = np.ascontiguousarray(
        np.asarray(bout, dtype=np.float32).reshape(KC, P).T)
    in_maps = []
    for c in range(N_CORES):
        sl = slice(c * SLICE, (c + 1) * SLICE)
        in_maps.append({
            "xTs": _bf16(x_f[sl].T),
            "cTs": _bf16(c_f[sl].T),
            "Wq": Wq, "Wk": Wk, "Wv": Wv, "Wout": Wout, "boutT": boutT,
        })
    return in_maps


def _assemble(results):
    y = np.empty((TOK, QD), dtype=np.float32)
    for c in range(N_CORES):
        yt = results[c]["yT"]   # [KC, P, SLICE]
        y[c * SLICE:(c + 1) * SLICE] = (
            yt.transpose(2, 0, 1).reshape(SLICE, QD))
    return y.reshape(2, TOK // 2, QD)


def run(inputs, trace=False, **kw):
    nc = _get_nc()
    in_maps = _prep_in_maps(**inputs)
    res = bass_utils.run_bass_kernel_spmd(
        nc, in_maps, core_ids=list(range(N_CORES)), trace=trace, **kw)
    return _assemble(res.results), res


def kernel(**inputs):
    out, _ = run(inputs, trace=False)
    return out
